# revision 54
# baseline (speedup 1.0000x reference)
"""FAPE loss Trainium2 kernel.

Math: for frames f (built from coord triples) and points n,
  d2[f,n] = ||Rp(p_n - po_f)||^2 + ||Rt(t_n - to_f)||^2 - 2 (p_n-po_f)^T M (t_n-to_f)
with M = Rp^T Rt.  Expanding, d2[f,n] = X[n] . Y[f] with 17 features:
  X = [A_n, 1, p (3), t (3), W (9)]   A_n = ||p_n||^2 + ||t_n||^2, W = outer(p_n, t_n)
  Y = [mask, B_f - 2c_f + off, 2(u-po), 2(v-to), -2M]  u = M to, v = M^T po,
      c_f = po.u, B_f = ||po||^2 + ||to||^2
Loss = mean(min(sqrt(d2 + eps), 10)) / 10.

The O(N) feature prep (X per point, Y per frame) is done host-side in numpy
and shipped pre-transposed in fp16 (tolerance is 2e-2; fp16 features give
~4e-4 end-to-end, same as the fp32r baseline whose DSQ_OFF bias dominates).
The device does the O(F*N) part:
  32 fp16 matmuls (K=17, 1 cycle/row -- ~4x the fp32r rate) -> PSUM f32 d2,
  then per supertile: ACT sqrt(d2+eps) -> bf16, DVE min-10 clamp (also
  squashes NaN from negative-noise d2), DVE tensor_add into a bf16
  [128,2048] running acc.  ACT is the stream pole at ~16.7us busy
  (0.833ns/col, no 16-bit fast mode), with DVE right behind; supertiles
  are tapered [512,1024,2048x6,1024,1024,512] so ACT starts early and the
  tail drains fast.  Tail: the LAST TWO supertiles skip the TT-add and
  ship their clamped values raw from the scalar queue (host sums them),
  so acc is final after tile 8 and ships fully hidden; only the last
  raw 128KB trails the compute.  Head: the 32 yt/window-0 rows supertile
  0 needs go out on the scalar DMA queue in parallel with the sync
  queue's chunks, landing in ~1us so the ACT stream starts at ~4.0us.
  All memsets run on GpSimd/DVE-idle time (an acc memset on DVE measured
  +1.7us on the critical tail).

Measured dead ends (HW): TensorTensorReduce faults the exec unit;
tensor_scalar/scalar_tensor_tensor accum_out lowers wrong or runs 1.3ns/col;
GpSimd cannot run TensorScalar ops at all and its TensorTensor add is
~2.1ns/col (7x DVE); raw-shipping clamped tiles to skip the TT-add loses
more to output-DMA tail than it saves; a DVE bit-hack sqrt lane (strided
u16 PSUM read + shift) works but inflates DVE past the ACT pole.

Sharding: frames split across 8 cores (512/core; the last core's 2 pad
frames have all-zero Y rows).  Points replicated.

Device layout per core:
  xt [96, 1408] f16: X^T in 11 windows of 128 cols (=128 points); window
      b, slot s in {0..2} holds feature k at partition 32s+k for point
      group g = 3b + s (points g*128 ..); 33rd group slot zero.
  yt [96, 512] f16: Y^T replicated at partition bases 0/32/64 so every
      lhsT slot finds a matching rhs.
"""
import sys

for _p in ("/opt/trn_rl_repo", "/root/.axon_site/_ro/trn_rl_repo"):
    if _p not in sys.path:
        sys.path.append(_p)

import numpy as np
from concourse import bass, bacc, mybir, tile
from concourse.bass_utils import run_bass_kernel_spmd

F32 = mybir.dt.float32
F16 = mybir.dt.float16
BF16 = mybir.dt.bfloat16
U16 = mybir.dt.uint16
AF = mybir.ActivationFunctionType
OP = mybir.AluOpType

N = 4096          # points
F = N - 2         # frames (4094)
NCORES = 8
FPC = 512         # frames per core (last core: 510 real + 2 zero-pad)
KF = 17           # contraction features
EPS = 1e-8
UNIT = 10.0
CLAMP = 10.0
DSQ_OFF = 1.0     # added to every real frame's d2 so fp16 noise can't push
                  # it far negative; ~3.9e-4 relative loss bias
NWIN = 11         # X^T windows of 128 points, 3 feature-slots each
STWIDTHS = [512, 1024] + [2048] * 6 + [1024, 1024, 512]  # tapered supertiles
NST = len(STWIDTHS)
# Tile 1 computes sqrt on DVE via the bf16 bit-hack instead of ACT: the
# ACT stream is the saturated pole (~16.7us) while DVE has ~2.8us slack,
# so dropping one 1024-col ACTIVATE compresses the whole stream.  The
# hack reads the high u16 of each PSUM f32 (= bf16 bits of d2) strided,
# shifts right 1 (halves the exponent ~ sqrt * 2^-63.45), then one
# dual-op tensor_scalar rescales by K and clamps at 10.  d2<0 noise
# shifts its sign bit into the exponent -> huge positive -> clamps to 10
# (same squash as the ACT path's NaN).  K tuned offline: ~1e-4 rel err
# on hacked elements, ~6% of all elements.
HACK_TILE = 1
HACK_K = 2.0 ** 63.4545


def build_nc():
    nc = bacc.Bacc(None)

    xy_d = nc.dram_tensor("xy", [96, 1920], F16, kind="ExternalInput")
    out_d = nc.dram_tensor("out", [128, 2048], BF16, kind="ExternalOutput")
    outt_d = nc.dram_tensor("outt", [128, 1536], BF16, kind="ExternalOutput")

    with tile.TileContext(nc) as tc:
        with (
            tc.tile_pool(name="inp", bufs=1) as inp,
            tc.tile_pool(name="sp", bufs=3) as sp,
            tc.tile_pool(name="tp", bufs=2) as tp,
            tc.tile_pool(name="accp", bufs=1) as accp,
            tc.tile_pool(name="psD", bufs=2, space="PSUM") as psD,
        ):
            xy_sb = inp.tile([96, 1920], F16)
            yt_sb = xy_sb[:, 0:FPC]
            xt_sb = xy_sb[:, FPC:1920]
            # staged input DMAs.  Supertiles 0-1 only read partitions
            # 0:96 of yt/window-0; the 32 rows supertile 0 needs (slot 0)
            # go out on the scalar queue in PARALLEL with the sync queue's
            # chunks, landing in ~0.6us instead of 1.9us, so the first
            # matmul -- and with it the whole ACT stream -- starts ~1.2us
            # earlier.  (Serializing the split on one queue measured
            # slower: each DMA issue costs ~0.65us on its queue.)
            nc.scalar.dma_start(xy_sb[0:32, 0:640], xy_d[0:32, 0:640])
            nc.sync.dma_start(xy_sb[32:96, 0:640], xy_d[32:96, 0:640])
            nc.sync.dma_start(xy_sb[:, 640:1280], xy_d[:, 640:1280])

            epst = inp.tile([128, 1], F32)
            nc.vector.memset(epst[:], EPS)

            # memset on the otherwise-idle GpSimd: on DVE this 1.76us
            # pushed DVE's total past the ACT pole and became the tail
            acc = accp.tile([128, 2048], BF16)
            nc.gpsimd.memset(acc[:], 0.0)

            g = 0
            for u, w in enumerate(STWIDTHS):
                nmm = w // FPC
                ps = psD.tile([128, 2048], F32, tag="d2")
                for h in range(nmm):
                    b, sl = divmod(g, 3)
                    g += 1
                    lhsT = xt_sb[32 * sl: 32 * sl + KF,
                                 b * 128: (b + 1) * 128]
                    rhs = yt_sb[32 * sl: 32 * sl + KF, 0:FPC]
                    nc.tensor.matmul(
                        ps[:, h * FPC: (h + 1) * FPC], lhsT, rhs,
                        start=True, stop=True,
                    )
                s = sp.tile([128, 2048], BF16, tag="s")
                tmp = tp.tile([128, 2048], BF16, tag="t")
                if u == HACK_TILE:
                    nc.vector.tensor_scalar(
                        s[:, 0:w].bitcast(U16),
                        ps[:, 0:w].bitcast(U16)[:, 1::2],
                        1, 0,
                        op0=OP.logical_shift_right,
                        op1=OP.logical_shift_right,
                    )
                    nc.vector.tensor_scalar(tmp[:, 0:w], s[:, 0:w],
                                            HACK_K, CLAMP,
                                            op0=OP.mult, op1=OP.min)
                else:
                    nc.scalar.activation(s[:, 0:w], ps[:, 0:w], AF.Sqrt,
                                         bias=epst[:])
                    # clamp on DVE in 16-bit fast mode; the min also
                    # squashes NaN from sqrt of negative-noise d2
                    nc.vector.tensor_scalar_min(tmp[:, 0:w], s[:, 0:w],
                                                CLAMP)
                if u >= NST - 2:
                    # last two tiles: skip the TT-add and ship their
                    # clamped values raw (host sums them) -- cuts the
                    # serial clamp->add->ship chain off the critical tail
                    off = 0 if u == NST - 2 else 1024
                    nc.scalar.dma_start(outt_d[:, off:off + w],
                                        tmp[:, 0:w])
                else:
                    nc.vector.tensor_add(acc[:, 0:w], acc[:, 0:w],
                                         tmp[:, 0:w])
                if u == 0:
                    # the last input chunk pipelines in behind supertile 0
                    nc.sync.dma_start(xy_sb[:, 1280:1920],
                                      xy_d[:, 1280:1920])
                elif u == 7:
                    # acc cols 1024:2048 final (later tiles are narrower)
                    nc.sync.dma_start(out_d[:, 1024:2048],
                                      acc[:, 1024:2048])
                elif u == NST - 3:
                    # tile 8 is the last acc writer: all of acc is final
                    # after its add; ship the rest fully hidden
                    nc.sync.dma_start(out_d[:, 0:1024], acc[:, 0:1024])

    nc.finalize()
    return nc


_NC_CACHE = None


def _get_nc():
    global _NC_CACHE
    if _NC_CACHE is None:
        _NC_CACHE = build_nc()
    return _NC_CACHE


def _frames(c):
    o = c[1:-1]
    e1 = c[2:] - c[1:-1]
    e1 = e1 / (np.linalg.norm(e1, axis=1, keepdims=True) + EPS)
    e2 = c[:-2] - c[1:-1]
    e2 = e2 - (e2 * e1).sum(1, keepdims=True) * e1
    e2 = e2 / (np.linalg.norm(e2, axis=1, keepdims=True) + EPS)
    e3 = np.cross(e1, e2)
    R = np.stack([e1, e2, e3], 1)          # [F,3,3], rows are basis vecs
    return o, R


def make_in_maps(pred_coords, true_coords):
    pred = np.ascontiguousarray(pred_coords, dtype=np.float32)
    true = np.ascontiguousarray(true_coords, dtype=np.float32)

    # X features [N, 17]
    A = (pred * pred).sum(1) + (true * true).sum(1)
    W = (pred[:, :, None] * true[:, None, :]).reshape(N, 9)
    X = np.concatenate(
        [A[:, None], np.ones((N, 1), np.float32), pred, true, W],
        axis=1).astype(np.float32)

    # Y features [F, 17]
    po, Rp = _frames(pred)
    to, Rt = _frames(true)
    M = np.einsum('frc,frd->fcd', Rp, Rt)      # Rp^T Rt
    u = np.einsum('fcd,fd->fc', M, to)
    v = np.einsum('fcd,fc->fd', M, po)
    cf = (po * u).sum(1)
    B = (po * po).sum(1) + (to * to).sum(1)
    Y = np.concatenate(
        [np.ones((F, 1), np.float32), (B - 2 * cf + DSQ_OFF)[:, None],
         2 * (u - po), 2 * (v - to), (-2 * M).reshape(F, 9)],
        axis=1).astype(np.float32)

    # X^T layout [96, 1408]: xt[32s + k, b*128 + c] = X[(3b + s)*128 + c, k]
    # (33rd group slot unused/zero); packed at cols 512:1920 of xy
    xt = np.zeros((96, 1408), np.float16)
    Xp = np.zeros((NWIN * 3 * 128, KF), np.float32)
    Xp[:N] = X
    tmp = Xp.reshape(NWIN, 3, 128, KF)         # [b, s, c, k]
    xt.reshape(3, 32, NWIN, 128)[:, :KF] = tmp.transpose(1, 3, 0, 2)

    in_maps = []
    for i in range(NCORES):
        f0 = i * FPC
        nvalid = min(FPC, F - f0)
        Yc = np.zeros((FPC, KF), np.float32)
        Yc[:nvalid] = Y[f0: f0 + nvalid]
        xy = np.zeros((96, 1920), np.float16)
        xy[:, FPC:1920] = xt
        xy.reshape(96, -1)[:, 0:FPC].reshape(3, 32, FPC)[:, :KF] = \
            Yc.T[None].astype(np.float16)
        in_maps.append({"xy": xy})
    return in_maps


def kernel(pred_coords, true_coords):
    nc = _get_nc()
    in_maps = make_in_maps(pred_coords, true_coords)
    res = run_bass_kernel_spmd(nc, in_maps, list(range(NCORES)))
    total = sum(float(np.asarray(r["out"], np.float32).sum()) +
                float(np.asarray(r["outt"], np.float32).sum())
                for r in res.results)
    return np.float32(total / (F * N) / UNIT)


# revision 55
# speedup vs baseline: 1.0057x; 1.0057x over previous
"""FAPE loss Trainium2 kernel.

Math: for frames f (built from coord triples) and points n,
  d2[f,n] = ||Rp(p_n - po_f)||^2 + ||Rt(t_n - to_f)||^2 - 2 (p_n-po_f)^T M (t_n-to_f)
with M = Rp^T Rt.  Expanding, d2[f,n] = X[n] . Y[f] with 17 features:
  X = [A_n, 1, p (3), t (3), W (9)]   A_n = ||p_n||^2 + ||t_n||^2, W = outer(p_n, t_n)
  Y = [mask, B_f - 2c_f + off, 2(u-po), 2(v-to), -2M]  u = M to, v = M^T po,
      c_f = po.u, B_f = ||po||^2 + ||to||^2
Loss = mean(min(sqrt(d2 + eps), 10)) / 10.

The O(N) feature prep (X per point, Y per frame) is done host-side in numpy
and shipped pre-transposed in fp16 (tolerance is 2e-2; fp16 features give
~4e-4 end-to-end, same as the fp32r baseline whose DSQ_OFF bias dominates).
The device does the O(F*N) part:
  32 fp16 matmuls (K=17, 1 cycle/row -- ~4x the fp32r rate) -> PSUM f32 d2,
  then per supertile: ACT sqrt(d2+eps) -> bf16, DVE min-10 clamp (also
  squashes NaN from negative-noise d2), DVE tensor_add into a bf16
  [128,2048] running acc.  ACT is the stream pole at ~16.7us busy
  (0.833ns/col, no 16-bit fast mode), with DVE right behind; supertiles
  are tapered [512,1024,2048x6,1024,1024,512] so ACT starts early and the
  tail drains fast.  Tail: the LAST TWO supertiles skip the TT-add and
  ship their clamped values raw from the scalar queue (host sums them),
  so acc is final after tile 8 and ships fully hidden; only the last
  raw 128KB trails the compute.  Head: the 32 yt/window-0 rows supertile
  0 needs go out on the scalar DMA queue in parallel with the sync
  queue's chunks, landing in ~1us so the ACT stream starts at ~4.0us.
  All memsets run on GpSimd/DVE-idle time (an acc memset on DVE measured
  +1.7us on the critical tail).

Measured dead ends (HW): TensorTensorReduce faults the exec unit;
tensor_scalar/scalar_tensor_tensor accum_out lowers wrong or runs 1.3ns/col;
GpSimd cannot run TensorScalar ops at all and its TensorTensor add is
~2.1ns/col (7x DVE); raw-shipping clamped tiles to skip the TT-add loses
more to output-DMA tail than it saves; a DVE bit-hack sqrt lane (strided
u16 PSUM read + shift) works but inflates DVE past the ACT pole.

Sharding: frames split across 8 cores (512/core; the last core's 2 pad
frames have all-zero Y rows).  Points replicated.

Device layout per core:
  xt [96, 1408] f16: X^T in 11 windows of 128 cols (=128 points); window
      b, slot s in {0..2} holds feature k at partition 32s+k for point
      group g = 3b + s (points g*128 ..); 33rd group slot zero.
  yt [96, 512] f16: Y^T replicated at partition bases 0/32/64 so every
      lhsT slot finds a matching rhs.
"""
import sys

for _p in ("/opt/trn_rl_repo", "/root/.axon_site/_ro/trn_rl_repo"):
    if _p not in sys.path:
        sys.path.append(_p)

import numpy as np
from concourse import bass, bacc, mybir, tile
from concourse.bass_utils import run_bass_kernel_spmd

F32 = mybir.dt.float32
F16 = mybir.dt.float16
BF16 = mybir.dt.bfloat16
U16 = mybir.dt.uint16
AF = mybir.ActivationFunctionType
OP = mybir.AluOpType

N = 4096          # points
F = N - 2         # frames (4094)
NCORES = 8
FPC = 512         # frames per core (last core: 510 real + 2 zero-pad)
KF = 17           # contraction features
EPS = 1e-8
UNIT = 10.0
CLAMP = 10.0
DSQ_OFF = 1.0     # added to every real frame's d2 so fp16 noise can't push
                  # it far negative; ~3.9e-4 relative loss bias
NWIN = 11         # X^T windows of 128 points, 3 feature-slots each
STWIDTHS = [512, 1024] + [2048] * 6 + [1024, 1024, 512]  # tapered supertiles
NST = len(STWIDTHS)


def build_nc():
    nc = bacc.Bacc(None)

    xy_d = nc.dram_tensor("xy", [96, 1920], F16, kind="ExternalInput")
    out_d = nc.dram_tensor("out", [128, 2048], BF16, kind="ExternalOutput")
    outt_d = nc.dram_tensor("outt", [128, 2560], BF16, kind="ExternalOutput")

    with tile.TileContext(nc) as tc:
        with (
            tc.tile_pool(name="inp", bufs=1) as inp,
            tc.tile_pool(name="sp", bufs=3) as sp,
            tc.tile_pool(name="tp", bufs=2) as tp,
            tc.tile_pool(name="accp", bufs=1) as accp,
            tc.tile_pool(name="psD", bufs=2, space="PSUM") as psD,
        ):
            xy_sb = inp.tile([96, 1920], F16)
            yt_sb = xy_sb[:, 0:FPC]
            xt_sb = xy_sb[:, FPC:1920]
            # staged input DMAs.  Supertiles 0-1 only read partitions
            # 0:96 of yt/window-0; the 32 rows supertile 0 needs (slot 0)
            # go out on the scalar queue in PARALLEL with the sync queue's
            # chunks, landing in ~0.6us instead of 1.9us, so the first
            # matmul -- and with it the whole ACT stream -- starts ~1.2us
            # earlier.  (Serializing the split on one queue measured
            # slower: each DMA issue costs ~0.65us on its queue.)
            nc.scalar.dma_start(xy_sb[0:32, 0:640], xy_d[0:32, 0:640])
            nc.sync.dma_start(xy_sb[32:96, 0:640], xy_d[32:96, 0:640])
            nc.sync.dma_start(xy_sb[:, 640:1280], xy_d[:, 640:1280])

            epst = inp.tile([128, 1], F32)
            nc.vector.memset(epst[:], EPS)

            # memset on the otherwise-idle GpSimd: on DVE this 1.76us
            # pushed DVE's total past the ACT pole and became the tail
            acc = accp.tile([128, 2048], BF16)
            nc.gpsimd.memset(acc[:], 0.0)

            g = 0
            for u, w in enumerate(STWIDTHS):
                nmm = w // FPC
                ps = psD.tile([128, 2048], F32, tag="d2")
                for h in range(nmm):
                    b, sl = divmod(g, 3)
                    g += 1
                    lhsT = xt_sb[32 * sl: 32 * sl + KF,
                                 b * 128: (b + 1) * 128]
                    rhs = yt_sb[32 * sl: 32 * sl + KF, 0:FPC]
                    nc.tensor.matmul(
                        ps[:, h * FPC: (h + 1) * FPC], lhsT, rhs,
                        start=True, stop=True,
                    )
                s = sp.tile([128, 2048], BF16, tag="s")
                # the last three tiles get DEDICATED tmp buffers: their
                # raw-ship DMAs hold the buffer, and a shared 2-deep pool
                # would stall the next tile's clamp on DMA completion
                tmp = tp.tile([128, 2048], BF16,
                              tag=f"t{u}" if u >= NST - 3 else "t")
                nc.scalar.activation(s[:, 0:w], ps[:, 0:w], AF.Sqrt,
                                     bias=epst[:])
                # clamp on DVE in 16-bit fast mode; the min also
                # squashes NaN from sqrt of negative-noise d2
                nc.vector.tensor_scalar_min(tmp[:, 0:w], s[:, 0:w],
                                            CLAMP)
                if u >= NST - 3:
                    # last three tiles: skip the TT-add and ship their
                    # clamped values raw (host sums them) -- cuts the
                    # serial clamp->add->ship chain off the critical tail
                    off = 1024 * (u - (NST - 3))
                    nc.scalar.dma_start(outt_d[:, off:off + w],
                                        tmp[:, 0:w])
                else:
                    nc.vector.tensor_add(acc[:, 0:w], acc[:, 0:w],
                                         tmp[:, 0:w])
                if u == 0:
                    # the last input chunk pipelines in behind supertile 0
                    nc.sync.dma_start(xy_sb[:, 1280:1920],
                                      xy_d[:, 1280:1920])
                elif u == 7:
                    # tile 7 is the last acc writer: acc is final after
                    # its add and ships whole, hidden behind tiles 8-10
                    nc.sync.dma_start(out_d[:], acc[:])

    nc.finalize()
    return nc


_NC_CACHE = None


def _get_nc():
    global _NC_CACHE
    if _NC_CACHE is None:
        _NC_CACHE = build_nc()
    return _NC_CACHE


def _frames(c):
    o = c[1:-1]
    e1 = c[2:] - c[1:-1]
    e1 = e1 / (np.linalg.norm(e1, axis=1, keepdims=True) + EPS)
    e2 = c[:-2] - c[1:-1]
    e2 = e2 - (e2 * e1).sum(1, keepdims=True) * e1
    e2 = e2 / (np.linalg.norm(e2, axis=1, keepdims=True) + EPS)
    e3 = np.cross(e1, e2)
    R = np.stack([e1, e2, e3], 1)          # [F,3,3], rows are basis vecs
    return o, R


def make_in_maps(pred_coords, true_coords):
    pred = np.ascontiguousarray(pred_coords, dtype=np.float32)
    true = np.ascontiguousarray(true_coords, dtype=np.float32)

    # X features [N, 17]
    A = (pred * pred).sum(1) + (true * true).sum(1)
    W = (pred[:, :, None] * true[:, None, :]).reshape(N, 9)
    X = np.concatenate(
        [A[:, None], np.ones((N, 1), np.float32), pred, true, W],
        axis=1).astype(np.float32)

    # Y features [F, 17]
    po, Rp = _frames(pred)
    to, Rt = _frames(true)
    M = np.einsum('frc,frd->fcd', Rp, Rt)      # Rp^T Rt
    u = np.einsum('fcd,fd->fc', M, to)
    v = np.einsum('fcd,fc->fd', M, po)
    cf = (po * u).sum(1)
    B = (po * po).sum(1) + (to * to).sum(1)
    Y = np.concatenate(
        [np.ones((F, 1), np.float32), (B - 2 * cf + DSQ_OFF)[:, None],
         2 * (u - po), 2 * (v - to), (-2 * M).reshape(F, 9)],
        axis=1).astype(np.float32)

    # X^T layout [96, 1408]: xt[32s + k, b*128 + c] = X[(3b + s)*128 + c, k]
    # (33rd group slot unused/zero); packed at cols 512:1920 of xy
    xt = np.zeros((96, 1408), np.float16)
    Xp = np.zeros((NWIN * 3 * 128, KF), np.float32)
    Xp[:N] = X
    tmp = Xp.reshape(NWIN, 3, 128, KF)         # [b, s, c, k]
    xt.reshape(3, 32, NWIN, 128)[:, :KF] = tmp.transpose(1, 3, 0, 2)

    in_maps = []
    for i in range(NCORES):
        f0 = i * FPC
        nvalid = min(FPC, F - f0)
        Yc = np.zeros((FPC, KF), np.float32)
        Yc[:nvalid] = Y[f0: f0 + nvalid]
        xy = np.zeros((96, 1920), np.float16)
        xy[:, FPC:1920] = xt
        xy.reshape(96, -1)[:, 0:FPC].reshape(3, 32, FPC)[:, :KF] = \
            Yc.T[None].astype(np.float16)
        in_maps.append({"xy": xy})
    return in_maps


def kernel(pred_coords, true_coords):
    nc = _get_nc()
    in_maps = make_in_maps(pred_coords, true_coords)
    res = run_bass_kernel_spmd(nc, in_maps, list(range(NCORES)))
    total = sum(float(np.asarray(r["out"], np.float32).sum()) +
                float(np.asarray(r["outt"], np.float32).sum())
                for r in res.results)
    return np.float32(total / (F * N) / UNIT)


# revision 56
# speedup vs baseline: 1.0654x; 1.0594x over previous
"""FAPE loss Trainium2 kernel.

Math: for frames f (built from coord triples) and points n,
  d2[f,n] = ||Rp(p_n - po_f)||^2 + ||Rt(t_n - to_f)||^2 - 2 (p_n-po_f)^T M (t_n-to_f)
with M = Rp^T Rt.  Expanding, d2[f,n] = X[n] . Y[f] with 17 features:
  X = [A_n, 1, p (3), t (3), W (9)]   A_n = ||p_n||^2 + ||t_n||^2, W = outer(p_n, t_n)
  Y = [mask, B_f - 2c_f + off, 2(u-po), 2(v-to), -2M]  u = M to, v = M^T po,
      c_f = po.u, B_f = ||po||^2 + ||to||^2
Loss = mean(min(sqrt(d2 + eps), 10)) / 10.

The O(N) feature prep (X per point, Y per frame) is done host-side in numpy
and shipped pre-transposed in fp16 (tolerance is 2e-2; fp16 features give
~4e-4 end-to-end, same as the fp32r baseline whose DSQ_OFF bias dominates).
The device does the O(F*N) part:
  32 fp16 matmuls (K=17, 1 cycle/row -- ~4x the fp32r rate) -> PSUM f32 d2,
  then per supertile: ACT sqrt(d2+eps) -> bf16, DVE min-10 clamp (also
  squashes NaN from negative-noise d2), DVE tensor_add into a bf16
  [128,2048] running acc.  ACT is the stream pole at ~16.7us busy
  (0.833ns/col, no 16-bit fast mode), with DVE right behind; supertiles
  are tapered [512,1024,2048x6,1024,1024,512] so ACT starts early and the
  tail drains fast.  Tail: the LAST TWO supertiles skip the TT-add and
  ship their clamped values raw from the scalar queue (host sums them),
  so acc is final after tile 8 and ships fully hidden; only the last
  raw 128KB trails the compute.  Head: the 32 yt/window-0 rows supertile
  0 needs go out on the scalar DMA queue in parallel with the sync
  queue's chunks, landing in ~1us so the ACT stream starts at ~4.0us.
  All memsets run on GpSimd/DVE-idle time (an acc memset on DVE measured
  +1.7us on the critical tail).

Measured dead ends (HW): TensorTensorReduce faults the exec unit;
tensor_scalar/scalar_tensor_tensor accum_out lowers wrong or runs 1.3ns/col;
GpSimd cannot run TensorScalar ops at all and its TensorTensor add is
~2.1ns/col (7x DVE); raw-shipping clamped tiles to skip the TT-add loses
more to output-DMA tail than it saves; a DVE bit-hack sqrt lane (strided
u16 PSUM read + shift) works but inflates DVE past the ACT pole.

Sharding: frames split across 8 cores (512/core; the last core's 2 pad
frames have all-zero Y rows).  Points replicated.

Device layout per core:
  xt [96, 1408] f16: X^T in 11 windows of 128 cols (=128 points); window
      b, slot s in {0..2} holds feature k at partition 32s+k for point
      group g = 3b + s (points g*128 ..); 33rd group slot zero.
  yt [96, 512] f16: Y^T replicated at partition bases 0/32/64 so every
      lhsT slot finds a matching rhs.
"""
import sys

for _p in ("/opt/trn_rl_repo", "/root/.axon_site/_ro/trn_rl_repo"):
    if _p not in sys.path:
        sys.path.append(_p)

import numpy as np
from concourse import bass, bacc, mybir, tile
from concourse.bass_utils import run_bass_kernel_spmd

F32 = mybir.dt.float32
F16 = mybir.dt.float16
BF16 = mybir.dt.bfloat16
U16 = mybir.dt.uint16
AF = mybir.ActivationFunctionType
OP = mybir.AluOpType

N = 4096          # points
F = N - 2         # frames (4094)
NCORES = 8
FPC = 512         # frames per core (last core: 510 real + 2 zero-pad)
KF = 17           # contraction features
EPS = 1e-8
UNIT = 10.0
CLAMP = 10.0
DSQ_OFF = 1.0     # added to every real frame's d2 so fp16 noise can't push
                  # it far negative; ~3.9e-4 relative loss bias
NWIN = 11         # X^T windows of 128 points, 3 feature-slots each
STWIDTHS = [512, 1024] + [2048] * 6 + [1024, 1024, 512]  # tapered supertiles
NST = len(STWIDTHS)


def build_nc():
    nc = bacc.Bacc(None)

    xy_d = nc.dram_tensor("xy", [96, 1920], F16, kind="ExternalInput")
    out_d = nc.dram_tensor("out", [128, 2048], BF16, kind="ExternalOutput")
    outt_d = nc.dram_tensor("outt", [128, 1536], BF16, kind="ExternalOutput")

    with tile.TileContext(nc) as tc:
        with (
            tc.tile_pool(name="inp", bufs=1) as inp,
            tc.tile_pool(name="sp", bufs=3) as sp,
            tc.tile_pool(name="tp", bufs=2) as tp,
            tc.tile_pool(name="accp", bufs=1) as accp,
            tc.tile_pool(name="psD", bufs=2, space="PSUM") as psD,
        ):
            xy_sb = inp.tile([96, 1920], F16)
            yt_sb = xy_sb[:, 0:FPC]
            xt_sb = xy_sb[:, FPC:1920]
            # staged input DMAs.  Supertiles 0-1 only read partitions
            # 0:96 of yt/window-0; the 32 rows supertile 0 needs (slot 0)
            # go out on the scalar queue in PARALLEL with the sync queue's
            # chunks, landing in ~0.6us instead of 1.9us, so the first
            # matmul -- and with it the whole ACT stream -- starts ~1.2us
            # earlier.  (Serializing the split on one queue measured
            # slower: each DMA issue costs ~0.65us on its queue.)
            nc.scalar.dma_start(xy_sb[0:32, 0:640], xy_d[0:32, 0:640])
            nc.sync.dma_start(xy_sb[32:96, 0:640], xy_d[32:96, 0:640])
            nc.sync.dma_start(xy_sb[:, 640:1280], xy_d[:, 640:1280])

            epst = inp.tile([128, 1], F32)
            nc.vector.memset(epst[:], EPS)

            # memset on the otherwise-idle GpSimd: on DVE this 1.76us
            # pushed DVE's total past the ACT pole and became the tail
            acc = accp.tile([128, 2048], BF16)
            nc.gpsimd.memset(acc[:], 0.0)

            g = 0
            for u, w in enumerate(STWIDTHS):
                nmm = w // FPC
                ps = psD.tile([128, 2048], F32, tag="d2")
                for h in range(nmm):
                    b, sl = divmod(g, 3)
                    g += 1
                    lhsT = xt_sb[32 * sl: 32 * sl + KF,
                                 b * 128: (b + 1) * 128]
                    rhs = yt_sb[32 * sl: 32 * sl + KF, 0:FPC]
                    nc.tensor.matmul(
                        ps[:, h * FPC: (h + 1) * FPC], lhsT, rhs,
                        start=True, stop=True,
                    )
                s = sp.tile([128, 2048], BF16, tag="s")
                # tile 8 gets a dedicated tmp buffer: its add is deferred
                # past tile 9 (below), so it must not alias the pool
                tmp = tp.tile([128, 2048], BF16,
                              tag="t8" if u == NST - 3 else "t")
                nc.scalar.activation(s[:, 0:w], ps[:, 0:w], AF.Sqrt,
                                     bias=epst[:])
                # clamp on DVE in 16-bit fast mode; the min also
                # squashes NaN from sqrt of negative-noise d2
                nc.vector.tensor_scalar_min(tmp[:, 0:w], s[:, 0:w],
                                            CLAMP)
                if u >= NST - 2:
                    # last two tiles: skip the TT-add and ship their
                    # clamped values raw (host sums them) -- cuts the
                    # serial clamp->add->ship chain off the critical tail
                    off = 0 if u == NST - 2 else 1024
                    nc.scalar.dma_start(outt_d[:, off:off + w],
                                        tmp[:, 0:w])
                elif u == NST - 3:
                    # defer tile 8's add: keep tile 9's clamp (emitted
                    # next) AHEAD of it in the in-order DVE queue, so
                    # tile 9's raw ship isn't stuck behind this add
                    tmp8 = tmp
                else:
                    nc.vector.tensor_add(acc[:, 0:w], acc[:, 0:w],
                                         tmp[:, 0:w])
                if u == 0:
                    # the last input chunk pipelines in behind supertile 0
                    nc.sync.dma_start(xy_sb[:, 1280:1920],
                                      xy_d[:, 1280:1920])
                elif u == 7:
                    # acc cols 1024:2048 final (later tiles are narrower)
                    nc.sync.dma_start(out_d[:, 1024:2048],
                                      acc[:, 1024:2048])
                elif u == NST - 2:
                    # tile 8's deferred add runs here, after tile 9's
                    # clamp; then acc is fully final -- ship the rest
                    nc.vector.tensor_add(acc[:, 0:1024], acc[:, 0:1024],
                                         tmp8[:, 0:1024])
                    nc.sync.dma_start(out_d[:, 0:1024], acc[:, 0:1024])

    nc.finalize()
    return nc


_NC_CACHE = None


def _get_nc():
    global _NC_CACHE
    if _NC_CACHE is None:
        _NC_CACHE = build_nc()
    return _NC_CACHE


def _frames(c):
    o = c[1:-1]
    e1 = c[2:] - c[1:-1]
    e1 = e1 / (np.linalg.norm(e1, axis=1, keepdims=True) + EPS)
    e2 = c[:-2] - c[1:-1]
    e2 = e2 - (e2 * e1).sum(1, keepdims=True) * e1
    e2 = e2 / (np.linalg.norm(e2, axis=1, keepdims=True) + EPS)
    e3 = np.cross(e1, e2)
    R = np.stack([e1, e2, e3], 1)          # [F,3,3], rows are basis vecs
    return o, R


def make_in_maps(pred_coords, true_coords):
    pred = np.ascontiguousarray(pred_coords, dtype=np.float32)
    true = np.ascontiguousarray(true_coords, dtype=np.float32)

    # X features [N, 17]
    A = (pred * pred).sum(1) + (true * true).sum(1)
    W = (pred[:, :, None] * true[:, None, :]).reshape(N, 9)
    X = np.concatenate(
        [A[:, None], np.ones((N, 1), np.float32), pred, true, W],
        axis=1).astype(np.float32)

    # Y features [F, 17]
    po, Rp = _frames(pred)
    to, Rt = _frames(true)
    M = np.einsum('frc,frd->fcd', Rp, Rt)      # Rp^T Rt
    u = np.einsum('fcd,fd->fc', M, to)
    v = np.einsum('fcd,fc->fd', M, po)
    cf = (po * u).sum(1)
    B = (po * po).sum(1) + (to * to).sum(1)
    Y = np.concatenate(
        [np.ones((F, 1), np.float32), (B - 2 * cf + DSQ_OFF)[:, None],
         2 * (u - po), 2 * (v - to), (-2 * M).reshape(F, 9)],
        axis=1).astype(np.float32)

    # X^T layout [96, 1408]: xt[32s + k, b*128 + c] = X[(3b + s)*128 + c, k]
    # (33rd group slot unused/zero); packed at cols 512:1920 of xy
    xt = np.zeros((96, 1408), np.float16)
    Xp = np.zeros((NWIN * 3 * 128, KF), np.float32)
    Xp[:N] = X
    tmp = Xp.reshape(NWIN, 3, 128, KF)         # [b, s, c, k]
    xt.reshape(3, 32, NWIN, 128)[:, :KF] = tmp.transpose(1, 3, 0, 2)

    in_maps = []
    for i in range(NCORES):
        f0 = i * FPC
        nvalid = min(FPC, F - f0)
        Yc = np.zeros((FPC, KF), np.float32)
        Yc[:nvalid] = Y[f0: f0 + nvalid]
        xy = np.zeros((96, 1920), np.float16)
        xy[:, FPC:1920] = xt
        xy.reshape(96, -1)[:, 0:FPC].reshape(3, 32, FPC)[:, :KF] = \
            Yc.T[None].astype(np.float16)
        in_maps.append({"xy": xy})
    return in_maps


def kernel(pred_coords, true_coords):
    nc = _get_nc()
    in_maps = make_in_maps(pred_coords, true_coords)
    res = run_bass_kernel_spmd(nc, in_maps, list(range(NCORES)))
    total = sum(float(np.asarray(r["out"], np.float32).sum()) +
                float(np.asarray(r["outt"], np.float32).sum())
                for r in res.results)
    return np.float32(total / (F * N) / UNIT)


# revision 57
# speedup vs baseline: 1.0765x; 1.0104x over previous
"""FAPE loss Trainium2 kernel.

Math: for frames f (built from coord triples) and points n,
  d2[f,n] = ||Rp(p_n - po_f)||^2 + ||Rt(t_n - to_f)||^2 - 2 (p_n-po_f)^T M (t_n-to_f)
with M = Rp^T Rt.  Expanding, d2[f,n] = X[n] . Y[f] with 17 features:
  X = [A_n, 1, p (3), t (3), W (9)]   A_n = ||p_n||^2 + ||t_n||^2, W = outer(p_n, t_n)
  Y = [mask, B_f - 2c_f + off, 2(u-po), 2(v-to), -2M]  u = M to, v = M^T po,
      c_f = po.u, B_f = ||po||^2 + ||to||^2
Loss = mean(min(sqrt(d2 + eps), 10)) / 10.

The O(N) feature prep (X per point, Y per frame) is done host-side in numpy
and shipped pre-transposed in fp16 (tolerance is 2e-2; fp16 features give
~4e-4 end-to-end, same as the fp32r baseline whose DSQ_OFF bias dominates).
The device does the O(F*N) part:
  32 fp16 matmuls (K=17, 1 cycle/row -- ~4x the fp32r rate) -> PSUM f32 d2,
  then per supertile: ACT sqrt(d2+eps) -> bf16, DVE min-10 clamp (also
  squashes NaN from negative-noise d2), DVE tensor_add into a bf16
  [128,2048] running acc.  ACT is the stream pole at ~16.7us busy
  (0.833ns/col, no 16-bit fast mode), with DVE right behind; supertiles
  are tapered [512,1024,2048x6,1024,1024,512] so ACT starts early and the
  tail drains fast.  Tail: the LAST TWO supertiles skip the TT-add and
  ship their clamped values raw from the scalar queue (host sums them),
  so acc is final after tile 8 and ships fully hidden; only the last
  raw 128KB trails the compute.  Head: the 32 yt/window-0 rows supertile
  0 needs go out on the scalar DMA queue in parallel with the sync
  queue's chunks, landing in ~1us so the ACT stream starts at ~4.0us.
  All memsets run on GpSimd/DVE-idle time (an acc memset on DVE measured
  +1.7us on the critical tail).

Measured dead ends (HW): TensorTensorReduce faults the exec unit;
tensor_scalar/scalar_tensor_tensor accum_out lowers wrong or runs 1.3ns/col;
GpSimd cannot run TensorScalar ops at all and its TensorTensor add is
~2.1ns/col (7x DVE); raw-shipping clamped tiles to skip the TT-add loses
more to output-DMA tail than it saves; a DVE bit-hack sqrt lane (strided
u16 PSUM read + shift) works but inflates DVE past the ACT pole.

Sharding: frames split across 8 cores (512/core; the last core's 2 pad
frames have all-zero Y rows).  Points replicated.

Device layout per core:
  xt [96, 1408] f16: X^T in 11 windows of 128 cols (=128 points); window
      b, slot s in {0..2} holds feature k at partition 32s+k for point
      group g = 3b + s (points g*128 ..); 33rd group slot zero.
  yt [96, 512] f16: Y^T replicated at partition bases 0/32/64 so every
      lhsT slot finds a matching rhs.
"""
import sys

for _p in ("/opt/trn_rl_repo", "/root/.axon_site/_ro/trn_rl_repo"):
    if _p not in sys.path:
        sys.path.append(_p)

import numpy as np
from concourse import bass, bacc, mybir, tile
from concourse.bass_utils import run_bass_kernel_spmd

F32 = mybir.dt.float32
F16 = mybir.dt.float16
BF16 = mybir.dt.bfloat16
U16 = mybir.dt.uint16
AF = mybir.ActivationFunctionType
OP = mybir.AluOpType

N = 4096          # points
F = N - 2         # frames (4094)
NCORES = 8
FPC = 512         # frames per core (last core: 510 real + 2 zero-pad)
KF = 17           # contraction features
EPS = 1e-8
UNIT = 10.0
CLAMP = 10.0
DSQ_OFF = 1.0     # added to every real frame's d2 so fp16 noise can't push
                  # it far negative; ~3.9e-4 relative loss bias
NWIN = 11         # X^T windows of 128 points, 3 feature-slots each
STWIDTHS = [512, 1024] + [2048] * 6 + [1024, 1024, 512]  # tapered supertiles
NST = len(STWIDTHS)


def build_nc():
    nc = bacc.Bacc(None)

    xy_d = nc.dram_tensor("xy", [96, 1920], F16, kind="ExternalInput")
    out_d = nc.dram_tensor("out", [128, 2048], BF16, kind="ExternalOutput")
    outt_d = nc.dram_tensor("outt", [128, 1536], BF16, kind="ExternalOutput")

    with tile.TileContext(nc) as tc:
        with (
            tc.tile_pool(name="inp", bufs=1) as inp,
            tc.tile_pool(name="sp", bufs=3) as sp,
            tc.tile_pool(name="tp", bufs=2) as tp,
            tc.tile_pool(name="accp", bufs=1) as accp,
            tc.tile_pool(name="psD", bufs=2, space="PSUM") as psD,
        ):
            xy_sb = inp.tile([96, 1920], F16)
            yt_sb = xy_sb[:, 0:FPC]
            xt_sb = xy_sb[:, FPC:1920]
            # staged input DMAs.  Supertiles 0-1 only read partitions
            # 0:96 of yt/window-0; the 32 rows supertile 0 needs (slot 0)
            # go out on the scalar queue in PARALLEL with the sync queue's
            # chunks, landing in ~0.6us instead of 1.9us, so the first
            # matmul -- and with it the whole ACT stream -- starts ~1.2us
            # earlier.  (Serializing the split on one queue measured
            # slower: each DMA issue costs ~0.65us on its queue.)
            nc.scalar.dma_start(xy_sb[0:32, 0:640], xy_d[0:32, 0:640])
            nc.sync.dma_start(xy_sb[32:96, 0:640], xy_d[32:96, 0:640])
            nc.sync.dma_start(xy_sb[:, 640:1280], xy_d[:, 640:1280])

            epst = inp.tile([128, 1], F32)
            nc.vector.memset(epst[:], EPS)

            # memset on the otherwise-idle GpSimd: on DVE this 1.76us
            # pushed DVE's total past the ACT pole and became the tail
            acc = accp.tile([128, 2048], BF16)
            nc.gpsimd.memset(acc[:], 0.0)

            g = 0
            for u, w in enumerate(STWIDTHS):
                nmm = w // FPC
                ps = psD.tile([128, 2048], F32, tag="d2")
                for h in range(nmm):
                    b, sl = divmod(g, 3)
                    g += 1
                    lhsT = xt_sb[32 * sl: 32 * sl + KF,
                                 b * 128: (b + 1) * 128]
                    rhs = yt_sb[32 * sl: 32 * sl + KF, 0:FPC]
                    nc.tensor.matmul(
                        ps[:, h * FPC: (h + 1) * FPC], lhsT, rhs,
                        start=True, stop=True,
                    )
                s = sp.tile([128, 2048], BF16, tag="s")
                tmp = tp.tile([128, 2048], BF16, tag="t")
                nc.scalar.activation(s[:, 0:w], ps[:, 0:w], AF.Sqrt,
                                     bias=epst[:])
                # clamp on DVE in 16-bit fast mode; the min also
                # squashes NaN from sqrt of negative-noise d2
                nc.vector.tensor_scalar_min(tmp[:, 0:w], s[:, 0:w],
                                            CLAMP)
                if u >= NST - 2:
                    # last two tiles: skip the TT-add and ship their
                    # clamped values raw (host sums them) -- cuts the
                    # serial clamp->add->ship chain off the critical tail
                    off = 0 if u == NST - 2 else 1024
                    nc.scalar.dma_start(outt_d[:, off:off + w],
                                        tmp[:, 0:w])
                else:
                    nc.vector.tensor_add(acc[:, 0:w], acc[:, 0:w],
                                         tmp[:, 0:w])
                if u == 0:
                    # the last input chunk pipelines in behind supertile 0
                    nc.sync.dma_start(xy_sb[:, 1280:1920],
                                      xy_d[:, 1280:1920])
                elif u == 7:
                    # acc cols 1024:2048 final (later tiles are narrower)
                    nc.sync.dma_start(out_d[:, 1024:2048],
                                      acc[:, 1024:2048])
                elif u == NST - 3:
                    # tile 8 is the last acc writer: all of acc is final
                    # after its add; ship the rest fully hidden
                    nc.sync.dma_start(out_d[:, 0:1024], acc[:, 0:1024])

    nc.finalize()
    return nc


_NC_CACHE = None


def _get_nc():
    global _NC_CACHE
    if _NC_CACHE is None:
        _NC_CACHE = build_nc()
    return _NC_CACHE


def _frames(c):
    o = c[1:-1]
    e1 = c[2:] - c[1:-1]
    e1 = e1 / (np.linalg.norm(e1, axis=1, keepdims=True) + EPS)
    e2 = c[:-2] - c[1:-1]
    e2 = e2 - (e2 * e1).sum(1, keepdims=True) * e1
    e2 = e2 / (np.linalg.norm(e2, axis=1, keepdims=True) + EPS)
    e3 = np.cross(e1, e2)
    R = np.stack([e1, e2, e3], 1)          # [F,3,3], rows are basis vecs
    return o, R


def make_in_maps(pred_coords, true_coords):
    pred = np.ascontiguousarray(pred_coords, dtype=np.float32)
    true = np.ascontiguousarray(true_coords, dtype=np.float32)

    # X features [N, 17]
    A = (pred * pred).sum(1) + (true * true).sum(1)
    W = (pred[:, :, None] * true[:, None, :]).reshape(N, 9)
    X = np.concatenate(
        [A[:, None], np.ones((N, 1), np.float32), pred, true, W],
        axis=1).astype(np.float32)

    # Y features [F, 17]
    po, Rp = _frames(pred)
    to, Rt = _frames(true)
    M = np.einsum('frc,frd->fcd', Rp, Rt)      # Rp^T Rt
    u = np.einsum('fcd,fd->fc', M, to)
    v = np.einsum('fcd,fc->fd', M, po)
    cf = (po * u).sum(1)
    B = (po * po).sum(1) + (to * to).sum(1)
    Y = np.concatenate(
        [np.ones((F, 1), np.float32), (B - 2 * cf + DSQ_OFF)[:, None],
         2 * (u - po), 2 * (v - to), (-2 * M).reshape(F, 9)],
        axis=1).astype(np.float32)

    # X^T layout [96, 1408]: xt[32s + k, b*128 + c] = X[(3b + s)*128 + c, k]
    # (33rd group slot unused/zero); packed at cols 512:1920 of xy
    xt = np.zeros((96, 1408), np.float16)
    Xp = np.zeros((NWIN * 3 * 128, KF), np.float32)
    Xp[:N] = X
    tmp = Xp.reshape(NWIN, 3, 128, KF)         # [b, s, c, k]
    xt.reshape(3, 32, NWIN, 128)[:, :KF] = tmp.transpose(1, 3, 0, 2)

    in_maps = []
    for i in range(NCORES):
        f0 = i * FPC
        nvalid = min(FPC, F - f0)
        Yc = np.zeros((FPC, KF), np.float32)
        Yc[:nvalid] = Y[f0: f0 + nvalid]
        xy = np.zeros((96, 1920), np.float16)
        xy[:, FPC:1920] = xt
        xy.reshape(96, -1)[:, 0:FPC].reshape(3, 32, FPC)[:, :KF] = \
            Yc.T[None].astype(np.float16)
        in_maps.append({"xy": xy})
    return in_maps


def kernel(pred_coords, true_coords):
    nc = _get_nc()
    in_maps = make_in_maps(pred_coords, true_coords)
    res = run_bass_kernel_spmd(nc, in_maps, list(range(NCORES)))
    total = sum(float(np.asarray(r["out"], np.float32).sum()) +
                float(np.asarray(r["outt"], np.float32).sum())
                for r in res.results)
    return np.float32(total / (F * N) / UNIT)
